# revision 4
# baseline (speedup 1.0000x reference)
"""Trainium2 Bass kernel for nn_DEC_26139170963600 (vq_codebook).

Reference computation:
  4x strided conv1d (stride 2, VALID) with LeakyReLU(0.1) between layers,
  flatten -> soft VQ assignment over 64 centers:
      d2 = ||z||^2 + ||c||^2 - 2 z.c
      q  = (1/(1+d2)) row-normalized            (alpha=1 -> exponent is 1)

Sharding: data-parallel over batch N=256 across 8 cores (32 samples/core).
Weights / centers replicated. No cross-device communication.

Per-core kernel design:
  - x stored in SBUF as (C=128 partitions, n*L) bf16, sample-major free dim.
  - conv layer = K tap-matmuls accumulated in PSUM:
        out[o, l] += W[o,:,k]^T . h[:, 2l+k]
    lhsT = W transposed to (i, o) per tap; rhs = strided slice of h.
    Later layers batch G samples per matmul (3D rhs AP) to keep the
    moving-operand free dim near 512 and amortize LDWEIGHTS.
  - PSUM eviction on ScalarE: LeakyReLU(psum + bias) in bf16. Implemented as
    relu(y) - relu(-0.1 y) (two ACT ops + one DVE subtract) unless USE_LRELU.
  - Distance: 59 bf16 matmuls accumulate -2 z.c into PSUM (32n x 64j);
    ||z||^2 via DVE square+reduce then fp32 matmul against ones;
    1 + ||c||^2 folded into a host-precomputed (32,64) fp32 tile.
  - q = reciprocal(d2+1) row-normalized on DVE, DMA out as fp32.
"""

import os
import sys

import numpy as np
import ml_dtypes

for _p in ("/opt/trn_rl_repo",):
    if _p not in sys.path and os.path.isdir(_p):
        sys.path.insert(0, _p)

import concourse.bacc as bacc  # noqa: E402
import concourse.mybir as mybir  # noqa: E402
import concourse.tile as tile  # noqa: E402
from concourse import bass_utils  # noqa: E402

HDT = mybir.dt.bfloat16  # NOTE: fp16 matmuls hard-fault trn2 here (NRT_EXEC_UNIT_UNRECOVERABLE)
F32 = mybir.dt.float32
AF = mybir.ActivationFunctionType
OP = mybir.AluOpType

N_CORES = 8
NS = 32          # samples per core
C = 128          # channels
KCENT = 64       # number of centers
LFIN = 59        # final length
D = C * LFIN     # 7552

# (K, L_in, L_out, G samples per matmul)
CFG = [
    (15, 1024, 505, 1),
    (12, 505, 247, 2),
    (7, 247, 121, 4),
    (4, 121, 59, 8),
]

USE_LRELU = True  # single fused ACT op; fall back to False if HW Lrelu is off

_BUILt = {}


def _build_program():
    """Build + compile the per-core Bass program (same program on all cores)."""
    nc = bacc.Bacc("TRN2", target_bir_lowering=False, debug=False)

    # ---- DRAM I/O ----
    x_d = nc.dram_tensor("x", (C, NS, 1024), HDT, kind="ExternalInput")
    w_d = [
        nc.dram_tensor(f"w{i+1}", (C, CFG[i][0] * C), HDT, kind="ExternalInput")
        for i in range(4)
    ]
    # bias pack: cols 0-3 = b1..b4; cols 4-6 = -0.1*b1..b3; col 7 = ones
    bp_d = nc.dram_tensor("bp", (C, 8), F32, kind="ExternalInput")
    cr_d = nc.dram_tensor("cr", (C, LFIN * KCENT), HDT, kind="ExternalInput")
    cnb_d = nc.dram_tensor("cnb", (NS, KCENT), F32, kind="ExternalInput")
    q_d = nc.dram_tensor("q", (NS, KCENT), F32, kind="ExternalOutput")

    with tile.TileContext(nc) as tc:
        with (
            tc.tile_pool(name="consts", bufs=1) as cpool,
            tc.tile_pool(name="xp", bufs=8) as xpool,
            tc.tile_pool(name="hp", bufs=1) as hpool,
            tc.tile_pool(name="sp", bufs=2) as spool,
            tc.tile_pool(name="small", bufs=1) as mpool,
            tc.tile_pool(name="psA", bufs=6, space="PSUM") as psA,
            tc.tile_pool(name="psZ", bufs=1, space="PSUM") as psZ,
            tc.tile_pool(name="psD", bufs=1, space="PSUM") as psD,
        ):
            # ---- constants to SBUF ----
            wt = []
            for i in range(4):
                t = cpool.tile([C, CFG[i][0] * C], HDT, tag=f"w{i}")
                nc.sync.dma_start(t[:], w_d[i].ap())
                wt.append(t)
            bp = cpool.tile([C, 8], F32, tag="bp")
            nc.sync.dma_start(bp[:], bp_d.ap())
            cr = cpool.tile([C, LFIN * KCENT], HDT, tag="cr")
            nc.sync.dma_start(cr[:], cr_d.ap())
            cnb = cpool.tile([NS, KCENT], F32, tag="cnb")
            nc.sync.dma_start(cnb[:], cnb_d.ap())

            # ---- x: 8 chunks of 4 samples ----
            xch = []
            for g in range(8):
                t = xpool.tile([C, 4 * 1024], HDT, tag="x")
                src = x_d.ap()[:, 4 * g : 4 * g + 4, :].rearrange("p a b -> p (a b)")
                nc.sync.dma_start(t[:], src)
                xch.append(t)

            # ---- conv stack ----
            h_tiles = []
            for li, (K, Lin, Lout, G) in enumerate(CFG):
                hdst = hpool.tile([C, NS * Lout], HDT, tag=f"h{li}")
                if li > 0:
                    hsrc3 = h_tiles[li - 1][:].rearrange("p (n l) -> p n l", n=NS)
                for g0 in range(0, NS, G):
                    ps = psA.tile([C, G * Lout], F32, tag="ps")
                    for k in range(K):
                        lhsT = wt[li][:, k * C : (k + 1) * C]
                        stop_idx = k + 2 * (Lout - 1) + 1
                        if li == 0:
                            x3 = xch[g0 // 4][:].rearrange("p (a b) -> p a b", a=4)
                            rhs = x3[:, g0 % 4 : g0 % 4 + 1, k : stop_idx : 2]
                        else:
                            rhs = hsrc3[:, g0 : g0 + G, k : stop_idx : 2]
                        nc.tensor.matmul(
                            ps[:], lhsT, rhs, start=(k == 0), stop=(k == K - 1)
                        )
                    dsl = hdst[:, g0 * Lout : (g0 + G) * Lout]
                    bias = bp[:, li : li + 1]
                    if li < 3:
                        if USE_LRELU:
                            nc.scalar.activation(
                                dsl, ps[:], AF.Lrelu, bias=bias, scale=1.0, alpha=0.1
                            )
                        else:
                            a = spool.tile([C, G * Lout], HDT, tag="a")
                            b2 = spool.tile([C, G * Lout], HDT, tag="b")
                            nbias = bp[:, 4 + li : 5 + li]
                            nc.scalar.activation(
                                a[:], ps[:], AF.Relu, bias=bias, scale=1.0
                            )
                            nc.scalar.activation(
                                b2[:], ps[:], AF.Relu, bias=nbias, scale=-0.1
                            )
                            nc.vector.tensor_tensor(dsl, a[:], b2[:], op=OP.subtract)
                    else:
                        nc.scalar.activation(
                            dsl, ps[:], AF.Identity, bias=bias, scale=1.0
                        )
                h_tiles.append(hdst)

            zb = h_tiles[3]  # (128, 32*59) bf16, sample-major

            # ---- ||z||^2 per sample ----
            zsq = hpool.tile([C, NS * LFIN], F32, tag="zsq")
            nc.vector.tensor_tensor(zsq[:], zb[:], zb[:], op=OP.mult)
            part = mpool.tile([C, NS], F32, tag="part")
            nc.vector.tensor_reduce(
                part[:],
                zsq[:].rearrange("p (n l) -> p n l", n=NS),
                axis=mybir.AxisListType.X,
                op=OP.add,
            )
            zn_ps = psZ.tile([NS, 1], F32, tag="zn")
            ones = bp[:, 7:8]
            nc.tensor.matmul(zn_ps[:], part[:], ones, start=True, stop=True)
            zn1 = mpool.tile([NS, 1], F32, tag="zn1")
            nc.scalar.copy(zn1[:], zn_ps[:])

            # ---- -2 z.c accumulated over 59 position-chunks ----
            d_ps = psD.tile([NS, KCENT], F32, tag="d")
            for l in range(LFIN):
                lhsT = zb[:, l : l + LFIN * (NS - 1) + 1 : LFIN]  # (128, 32)
                rhs = cr[:, l * KCENT : (l + 1) * KCENT]  # (128, 64)
                nc.tensor.matmul(
                    d_ps[:], lhsT, rhs, start=(l == 0), stop=(l == LFIN - 1)
                )

            # ---- q = normalize(1/(1+d2)) ----
            t1 = mpool.tile([NS, KCENT], F32, tag="t1")
            nc.vector.tensor_scalar_add(t1[:], d_ps[:], zn1[:])
            nc.vector.tensor_tensor(t1[:], t1[:], cnb[:], op=OP.add)
            qn = mpool.tile([NS, KCENT], F32, tag="qn")
            nc.vector.reciprocal(qn[:], t1[:])
            rs = mpool.tile([NS, 1], F32, tag="rs")
            nc.vector.tensor_reduce(
                rs[:], qn[:], axis=mybir.AxisListType.X, op=OP.add
            )
            rr = mpool.tile([NS, 1], F32, tag="rr")
            nc.vector.reciprocal(rr[:], rs[:])
            nc.vector.tensor_scalar_mul(qn[:], qn[:], rr[:])
            nc.sync.dma_start(q_d.ap(), qn[:])

    nc.compile()
    return nc


def _get_program():
    if "nc" not in _BUILt:
        _BUILt["nc"] = _build_program()
    return _BUILt["nc"]


def _prep_inputs(x, w1, b1, w2, b2, w3, b3, w4, b4, centers):
    """Host-side prep: dtype casts, weight transposes, per-core sharding."""
    ws = [w1, w2, w3, w4]
    bs = [b1, b2, b3, b4]

    const_map = {}
    for i, w in enumerate(ws):
        K = CFG[i][0]
        # (O, I, K) -> (I, K, O) -> (128, K*128); lhsT tap k = [:, k*128:(k+1)*128]
        const_map[f"w{i+1}"] = np.ascontiguousarray(
            np.asarray(w, np.float32).transpose(1, 2, 0).reshape(C, K * C)
        ).astype(ml_dtypes.bfloat16)

    bp = np.zeros((C, 8), np.float32)
    for i, b in enumerate(bs):
        bp[:, i] = np.asarray(b, np.float32)
    for i in range(3):
        bp[:, 4 + i] = -0.1 * np.asarray(bs[i], np.float32)
    bp[:, 7] = 1.0
    const_map["bp"] = bp

    cent = np.asarray(centers, np.float32)
    # cr[c, l*64 + j] = -2 * centers[j, c*59 + l]
    const_map["cr"] = np.ascontiguousarray(
        (-2.0 * cent).reshape(KCENT, C, LFIN).transpose(1, 2, 0).reshape(C, LFIN * KCENT)
    ).astype(ml_dtypes.bfloat16)
    cn = 1.0 + (cent.astype(np.float64) ** 2).sum(axis=1)  # (64,)
    const_map["cnb"] = np.broadcast_to(
        cn.astype(np.float32)[None, :], (NS, KCENT)
    ).copy()

    xf = np.asarray(x, np.float32)
    in_maps = []
    for c in range(N_CORES):
        shard = xf[c * NS : (c + 1) * NS]  # (32, 128, 1024)
        xc = np.ascontiguousarray(shard.transpose(1, 0, 2)).astype(ml_dtypes.bfloat16)  # (128,32,1024)
        in_maps.append({"x": xc, **const_map})
    return in_maps


def run(trace=False, **inputs):
    """Run the kernel; returns (q_full, BassKernelResults)."""
    nc = _get_program()
    in_maps = _prep_inputs(**inputs)
    res = bass_utils.run_bass_kernel_spmd(
        nc, in_maps, core_ids=list(range(N_CORES)), trace=trace
    )
    q = np.concatenate([res.results[c]["q"] for c in range(N_CORES)], axis=0)
    return np.ascontiguousarray(q.astype(np.float32)), res


def kernel(**inputs) -> np.ndarray:
    q, _ = run(trace=False, **inputs)
    return q


# revision 5
# speedup vs baseline: 2.8050x; 2.8050x over previous
"""Trainium2 Bass kernel for nn_DEC_26139170963600 (vq_codebook).

Reference computation:
  4x strided conv1d (stride 2, VALID) with LeakyReLU(0.1) between layers,
  flatten -> soft VQ assignment over 64 centers:
      d2 = ||z||^2 + ||c||^2 - 2 z.c
      q  = (1/(1+d2)) row-normalized            (alpha=1 -> exponent is 1)

Sharding: data-parallel over batch N=256 across 8 cores (32 samples/core).
Weights / centers replicated. No cross-device communication.

Per-core kernel design:
  - x stored in SBUF as (C=128 partitions, n*L) bf16, sample-major free dim.
  - conv layer = K tap-matmuls accumulated in PSUM:
        out[o, l] += W[o,:,k]^T . h[:, 2l+k]
    lhsT = W transposed to (i, o) per tap; rhs = strided slice of h.
    Later layers batch G samples per matmul (3D rhs AP) to keep the
    moving-operand free dim near 512 and amortize LDWEIGHTS.
  - PSUM eviction on ScalarE: LeakyReLU(psum + bias) in bf16. Implemented as
    relu(y) - relu(-0.1 y) (two ACT ops + one DVE subtract) unless USE_LRELU.
  - Distance: 59 bf16 matmuls accumulate -2 z.c into PSUM (32n x 64j);
    ||z||^2 via DVE square+reduce then fp32 matmul against ones;
    1 + ||c||^2 folded into a host-precomputed (32,64) fp32 tile.
  - q = reciprocal(d2+1) row-normalized on DVE, DMA out as fp32.
"""

import os
import sys

import numpy as np
import ml_dtypes

for _p in ("/opt/trn_rl_repo",):
    if _p not in sys.path and os.path.isdir(_p):
        sys.path.insert(0, _p)

import concourse.bacc as bacc  # noqa: E402
import concourse.mybir as mybir  # noqa: E402
import concourse.tile as tile  # noqa: E402
from concourse import bass_utils  # noqa: E402

HDT = mybir.dt.bfloat16  # NOTE: fp16 matmuls hard-fault trn2 here (NRT_EXEC_UNIT_UNRECOVERABLE)
F32 = mybir.dt.float32
AF = mybir.ActivationFunctionType
OP = mybir.AluOpType

N_CORES = 8
NS = 32          # samples per core
C = 128          # channels
KCENT = 64       # number of centers
LFIN = 59        # final length
D = C * LFIN     # 7552

# (K, L_in, L_out, G samples per matmul)
CFG = [
    (15, 1024, 505, 1),
    (12, 505, 247, 2),
    (7, 247, 121, 4),
    (4, 121, 59, 8),
]

USE_LRELU = True  # single fused ACT op; fall back to False if HW Lrelu is off

_BUILt = {}


def _build_program(n_repeat=1):
    """Build + compile the per-core Bass program (same program on all cores).

    n_repeat > 1 unrolls the full per-inference body that many times inside
    one NEFF (constants loaded once) — used only for slope timing in bench.py.
    """
    nc = bacc.Bacc("TRN2", target_bir_lowering=False, debug=False)

    # ---- DRAM I/O ----
    x_d = nc.dram_tensor("x", (C, NS, 1024), HDT, kind="ExternalInput")
    w_d = [
        nc.dram_tensor(f"w{i+1}", (C, CFG[i][0] * C), HDT, kind="ExternalInput")
        for i in range(4)
    ]
    # bias pack: cols 0-3 = b1..b4; cols 4-6 = -0.1*b1..b3; col 7 = ones
    bp_d = nc.dram_tensor("bp", (C, 8), F32, kind="ExternalInput")
    cr_d = nc.dram_tensor("cr", (C, LFIN * KCENT), HDT, kind="ExternalInput")
    cnb_d = nc.dram_tensor("cnb", (NS, KCENT), F32, kind="ExternalInput")
    q_d = nc.dram_tensor("q", (NS, KCENT), F32, kind="ExternalOutput")

    with tile.TileContext(nc) as tc:
        with (
            tc.tile_pool(name="consts", bufs=1) as cpool,
            tc.tile_pool(name="xp", bufs=8) as xpool,
            tc.tile_pool(name="hp", bufs=1) as hpool,
            tc.tile_pool(name="sp", bufs=2) as spool,
            tc.tile_pool(name="small", bufs=1) as mpool,
            tc.tile_pool(name="psA", bufs=6, space="PSUM") as psA,
            tc.tile_pool(name="psZ", bufs=1, space="PSUM") as psZ,
            tc.tile_pool(name="psD", bufs=1, space="PSUM") as psD,
        ):
            # ---- constants to SBUF ----
            wt = []
            for i in range(4):
                t = cpool.tile([C, CFG[i][0] * C], HDT, tag=f"w{i}")
                nc.sync.dma_start(t[:], w_d[i].ap())
                wt.append(t)
            bp = cpool.tile([C, 8], F32, tag="bp")
            nc.sync.dma_start(bp[:], bp_d.ap())
            cr = cpool.tile([C, LFIN * KCENT], HDT, tag="cr")
            nc.sync.dma_start(cr[:], cr_d.ap())
            cnb = cpool.tile([NS, KCENT], F32, tag="cnb")
            nc.sync.dma_start(cnb[:], cnb_d.ap())

            for _rep in range(n_repeat):
                _body_once(nc, tc, x_d, q_d, wt, bp, cr, cnb, xpool, hpool,
                           spool, mpool, psA, psZ, psD)

    nc.compile()
    return nc


def _body_once(nc, tc, x_d, q_d, wt, bp, cr, cnb, xpool, hpool, spool, mpool,
               psA, psZ, psD):
            # ---- x: 8 chunks of 4 samples ----
            xch = []
            for g in range(8):
                t = xpool.tile([C, 4 * 1024], HDT, tag="x")
                src = x_d.ap()[:, 4 * g : 4 * g + 4, :].rearrange("p a b -> p (a b)")
                nc.sync.dma_start(t[:], src)
                xch.append(t)

            # ---- conv stack ----
            h_tiles = []
            for li, (K, Lin, Lout, G) in enumerate(CFG):
                hdst = hpool.tile([C, NS * Lout], HDT, tag=f"h{li}")
                if li > 0:
                    hsrc3 = h_tiles[li - 1][:].rearrange("p (n l) -> p n l", n=NS)
                for g0 in range(0, NS, G):
                    ps = psA.tile([C, G * Lout], F32, tag="ps")
                    for k in range(K):
                        lhsT = wt[li][:, k * C : (k + 1) * C]
                        stop_idx = k + 2 * (Lout - 1) + 1
                        if li == 0:
                            x3 = xch[g0 // 4][:].rearrange("p (a b) -> p a b", a=4)
                            rhs = x3[:, g0 % 4 : g0 % 4 + 1, k : stop_idx : 2]
                        else:
                            rhs = hsrc3[:, g0 : g0 + G, k : stop_idx : 2]
                        nc.tensor.matmul(
                            ps[:], lhsT, rhs, start=(k == 0), stop=(k == K - 1)
                        )
                    dsl = hdst[:, g0 * Lout : (g0 + G) * Lout]
                    bias = bp[:, li : li + 1]
                    if li < 3:
                        if USE_LRELU:
                            nc.scalar.activation(
                                dsl, ps[:], AF.Lrelu, bias=bias, scale=1.0, alpha=0.1
                            )
                        else:
                            a = spool.tile([C, G * Lout], HDT, tag="a")
                            b2 = spool.tile([C, G * Lout], HDT, tag="b")
                            nbias = bp[:, 4 + li : 5 + li]
                            nc.scalar.activation(
                                a[:], ps[:], AF.Relu, bias=bias, scale=1.0
                            )
                            nc.scalar.activation(
                                b2[:], ps[:], AF.Relu, bias=nbias, scale=-0.1
                            )
                            nc.vector.tensor_tensor(dsl, a[:], b2[:], op=OP.subtract)
                    else:
                        nc.scalar.activation(
                            dsl, ps[:], AF.Identity, bias=bias, scale=1.0
                        )
                h_tiles.append(hdst)

            zb = h_tiles[3]  # (128, 32*59) bf16, sample-major

            # ---- ||z||^2 per sample ----
            zsq = hpool.tile([C, NS * LFIN], F32, tag="zsq")
            nc.vector.tensor_tensor(zsq[:], zb[:], zb[:], op=OP.mult)
            part = mpool.tile([C, NS], F32, tag="part")
            nc.vector.tensor_reduce(
                part[:],
                zsq[:].rearrange("p (n l) -> p n l", n=NS),
                axis=mybir.AxisListType.X,
                op=OP.add,
            )
            zn_ps = psZ.tile([NS, 1], F32, tag="zn")
            ones = bp[:, 7:8]
            nc.tensor.matmul(zn_ps[:], part[:], ones, start=True, stop=True)
            zn1 = mpool.tile([NS, 1], F32, tag="zn1")
            nc.scalar.copy(zn1[:], zn_ps[:])

            # ---- -2 z.c accumulated over 59 position-chunks ----
            d_ps = psD.tile([NS, KCENT], F32, tag="d")
            for l in range(LFIN):
                lhsT = zb[:, l : l + LFIN * (NS - 1) + 1 : LFIN]  # (128, 32)
                rhs = cr[:, l * KCENT : (l + 1) * KCENT]  # (128, 64)
                nc.tensor.matmul(
                    d_ps[:], lhsT, rhs, start=(l == 0), stop=(l == LFIN - 1)
                )

            # ---- q = normalize(1/(1+d2)) ----
            t1 = mpool.tile([NS, KCENT], F32, tag="t1")
            nc.vector.tensor_scalar_add(t1[:], d_ps[:], zn1[:])
            nc.vector.tensor_tensor(t1[:], t1[:], cnb[:], op=OP.add)
            qn = mpool.tile([NS, KCENT], F32, tag="qn")
            nc.vector.reciprocal(qn[:], t1[:])
            rs = mpool.tile([NS, 1], F32, tag="rs")
            nc.vector.tensor_reduce(
                rs[:], qn[:], axis=mybir.AxisListType.X, op=OP.add
            )
            rr = mpool.tile([NS, 1], F32, tag="rr")
            nc.vector.reciprocal(rr[:], rs[:])
            nc.vector.tensor_scalar_mul(qn[:], qn[:], rr[:])
            nc.sync.dma_start(q_d.ap(), qn[:])


def _get_program(n_repeat=1):
    if n_repeat not in _BUILt:
        _BUILt[n_repeat] = _build_program(n_repeat)
    return _BUILt[n_repeat]


def _prep_inputs(x, w1, b1, w2, b2, w3, b3, w4, b4, centers):
    """Host-side prep: dtype casts, weight transposes, per-core sharding."""
    ws = [w1, w2, w3, w4]
    bs = [b1, b2, b3, b4]

    const_map = {}
    for i, w in enumerate(ws):
        K = CFG[i][0]
        # (O, I, K) -> (I, K, O) -> (128, K*128); lhsT tap k = [:, k*128:(k+1)*128]
        const_map[f"w{i+1}"] = np.ascontiguousarray(
            np.asarray(w, np.float32).transpose(1, 2, 0).reshape(C, K * C)
        ).astype(ml_dtypes.bfloat16)

    bp = np.zeros((C, 8), np.float32)
    for i, b in enumerate(bs):
        bp[:, i] = np.asarray(b, np.float32)
    for i in range(3):
        bp[:, 4 + i] = -0.1 * np.asarray(bs[i], np.float32)
    bp[:, 7] = 1.0
    const_map["bp"] = bp

    cent = np.asarray(centers, np.float32)
    # cr[c, l*64 + j] = -2 * centers[j, c*59 + l]
    const_map["cr"] = np.ascontiguousarray(
        (-2.0 * cent).reshape(KCENT, C, LFIN).transpose(1, 2, 0).reshape(C, LFIN * KCENT)
    ).astype(ml_dtypes.bfloat16)
    cn = 1.0 + (cent.astype(np.float64) ** 2).sum(axis=1)  # (64,)
    const_map["cnb"] = np.broadcast_to(
        cn.astype(np.float32)[None, :], (NS, KCENT)
    ).copy()

    xf = np.asarray(x, np.float32)
    in_maps = []
    for c in range(N_CORES):
        shard = xf[c * NS : (c + 1) * NS]  # (32, 128, 1024)
        xc = np.ascontiguousarray(shard.transpose(1, 0, 2)).astype(ml_dtypes.bfloat16)  # (128,32,1024)
        in_maps.append({"x": xc, **const_map})
    return in_maps


def run(trace=False, **inputs):
    """Run the kernel; returns (q_full, BassKernelResults)."""
    nc = _get_program()
    in_maps = _prep_inputs(**inputs)
    res = bass_utils.run_bass_kernel_spmd(
        nc, in_maps, core_ids=list(range(N_CORES)), trace=trace
    )
    q = np.concatenate([res.results[c]["q"] for c in range(N_CORES)], axis=0)
    return np.ascontiguousarray(q.astype(np.float32)), res


def kernel(**inputs) -> np.ndarray:
    q, _ = run(trace=False, **inputs)
    return q
